# revision 14
# baseline (speedup 1.0000x reference)
"""GNN message-passing (EGNN-style classifier) on 8 TRN2 NeuronCores.

Data-parallel over ligands: each core handles 128 ligands = 4096 nodes,
32768 edges (edges never cross ligands). Weights replicated.

Dispatch: the jitted shard_map executor is built once and cached
(rebuilding re-traces and re-lowers the program, ~2s/call), and all
per-core inputs travel in one packed bf16 DRAM blob plus one int8 blob
(the axon PJRT path pays a large fixed cost per input tensor per device,
and the ~1Gbps tunnel makes payload bytes the dominant cost). Payload
diet: Gaussian smearing is computed on device from bf16 hi+lo distance
rows (hi+lo reconstructs f32), the col-gather one-hot and the R/S
scatter patterns are built on device from iotas / packed indices, the
time-bond embedding travels as per-feature int8 codes whose scales are
folded into the C weight rows (quantization error ~= bf16 rounding),
and W_f travels as bf16 hi+lo pairs.

Device layout (per core):
- Node state hh kept feature-major [128 feats, 4096 nodes] in SBUF (f32 master
  + bf16 copy for matmul inputs).
- Edge pipeline per layer, per group of 1024 edges (8 chunks x 128 edges):
  m1_pre = hh[row] @ A + hh[col] @ B + edge_attr @ C computed edge-major via
  three PE matmuls per chunk (R-gather / one-hot gather / edge-attr lhsT).
  LayerNorm stats via DVE bn_stats on PSUM; fused scale/bias+SiLU on ACT
  (edge-major -> per-partition scalars). DMA-transpose to feature-major,
  We2 matmul, SiLU, attention via PE (Watt column / mij_fm lhsT), gated
  segment-sum via one-hot-weighted (S*att) matmuls back to node-major.
- Node MLP node-major with the same LN trick; residual update in f32.
- The col-gather one-hot [128, 8192] is built on device from 32KB of
  packed per-band int8 indices: PE broadcast (band-selector matmul) +
  DVE is_equal against a partition iota.
- t_ea rows per half: 0:20 smear (ACT, quadrant-aligned), 20:32 pad
  (coeff=0 -> exp(0)=1, zero C rows), 32:48 int8 emb codes (aligned DVE
  convert); the edge-attr matmul contracts over 48 rows.
"""
import numpy as np
import ml_dtypes

N_LIG = 1024
K = 32                 # atoms per ligand
N = N_LIG * K          # 32768 nodes
KNN = 8
E = N * KNN            # 262144 edges
IN_F = 16
T_F = 16
HID = 128
OUT_F = 64
DEPTH = 4
NG = 20
NT = 1000
EDGE_IN = NG + T_F
NORM_FACTOR = 5.0
EPS = 1e-5

NCORES = 8
NLc = N // NCORES      # 4096 nodes / core
NEc = E // NCORES      # 32768 edges / core
LIGc = N_LIG // NCORES  # 128 ligands / core
NCHUNK = NEc // 128    # 256 edge chunks / core
NGRP = NCHUNK // 8     # 32 groups of 1024 edges

bf16 = ml_dtypes.bfloat16

# Gaussian smearing constants
_off = np.exp(np.linspace(np.log(1.0), np.log(5.0), NG)) - 1.0
_d = np.diff(_off)
_d = np.concatenate([_d[:1], _d])
GS_OFFSET = _off.astype(np.float32)
GS_COEFF = (-0.5 / _d ** 2).astype(np.float32)

# ---------------- blob layout (elements, bf16) ----------------
_OFF = {}
_tot = 0


def _lay(name, n):
    global _tot
    _OFF[name] = _tot
    _tot += n


_lay("dhl", 2 * NEc)            # clipped edge distance, bf16 hi+lo rows
_lay("gs", 4 * 32)              # [-offset, coeff] hi/lo pairs, [32,2]+[32,2]
_lay("infm", 32 * NLc)
_lay("Aaug", 128 * DEPTH * 129)
_lay("Baug", 128 * DEPTH * 129)
_lay("C", 48 * DEPTH * 129)
_lay("We2", 128 * DEPTH * 128)
_lay("Watt", 128 * DEPTH)
_lay("N1", 128 * DEPTH * 2 * 129)
_lay("Wn2", 128 * DEPTH * 128)
_lay("Win", 32 * 128)
_lay("Woe", 128 * 64)
_lay("pool", 128 * 4)
_lay("Wfhl", 64 * 2)            # f32 W_f as bf16 hi+lo pairs
TOTAL = _tot

# int8 side-blob: per-feature-quantized time-emb codes + one-hot indices
_OFF8 = {"emb8": 0, "v48": 16 * NEc}
TOTAL8 = 16 * NEc + 4 * 8192

_COMPILED = {}


def _build_program():
    import concourse.bacc as bacc
    import concourse.bass as bass
    import concourse.mybir as mybir
    import concourse.tile as tile

    bf = mybir.dt.bfloat16
    f32 = mybir.dt.float32
    i32 = mybir.dt.int32
    AF = mybir.ActivationFunctionType
    ALU = mybir.AluOpType

    nc = bacc.Bacc("TRN2", target_bir_lowering=False, debug=False)

    i8 = mybir.dt.int8
    d_blob = nc.dram_tensor("blob", [1, TOTAL], bf, kind="ExternalInput")
    d_blob8 = nc.dram_tensor("blob8", [1, TOTAL8], i8, kind="ExternalInput")
    d_out = nc.dram_tensor("out", [1, LIGc], f32, kind="ExternalOutput")

    def bv(name, ap, extra_off=0):
        return bass.AP(tensor=d_blob[:].tensor, offset=_OFF[name] + extra_off,
                       ap=ap)

    def bv8(name, ap, extra_off=0):
        return bass.AP(tensor=d_blob8[:].tensor, offset=_OFF8[name] + extra_off,
                       ap=ap)

    with tile.TileContext(nc) as tc:
        with tc.tile_pool(name="stat", bufs=1) as stat, \
             tc.tile_pool(name="hhp", bufs=1) as hhp, \
             tc.tile_pool(name="stg", bufs=4) as stg, \
             tc.tile_pool(name="sml", bufs=4) as sml, \
             tc.tile_pool(name="ps1", bufs=2, space="PSUM") as ps1, \
             tc.tile_pool(name="ps2", bufs=1, space="PSUM") as ps2, \
             tc.tile_pool(name="ps3", bufs=2, space="PSUM") as ps3:

            # ---------- static loads (strided views of the blob) ----------
            t_ea = stat.tile([128, 16384], bf, tag="t_ea")
            for ch in range(32):
                e8 = sml.tile([48, 1024], i8, tag="e8")
                nc.sync.dma_start(e8[32:48, :],
                                  bv8("emb8", [[NEc, 16], [1, 1024]], 1024 * ch))
                hf, cw = ch // 16, ch % 16
                nc.vector.tensor_copy(
                    t_ea[64 * hf + 32:64 * hf + 48, 1024 * cw:1024 * cw + 1024],
                    e8[32:48, :])
            t_gshl = stat.tile([32, 4], bf, tag="t_gshl")
            nc.sync.dma_start(t_gshl[:], bv("gs", [[4, 32], [1, 4]]))
            t_v4 = stat.tile([4, 8192], bf, tag="t_v4")
            for ch in range(8):
                e8 = sml.tile([48, 1024], i8, tag="e8")
                nc.sync.dma_start(e8[0:4, :],
                                  bv8("v48", [[8192, 4], [1, 1024]], 1024 * ch))
                nc.vector.tensor_copy(t_v4[:, 1024 * ch:1024 * ch + 1024],
                                      e8[0:4, :])
            t_R = stat.tile([128, 256], bf, tag="t_R")
            t_S = stat.tile([128, 256], bf, tag="t_S")
            t_in = stat.tile([32, NLc], bf, tag="t_in")
            nc.sync.dma_start(t_in[:], bv("infm", [[NLc, 32], [1, NLc]]))
            t_Aaug = stat.tile([128, DEPTH, 129], bf, tag="t_Aaug")
            nc.sync.dma_start(t_Aaug[:],
                              bv("Aaug", [[516, 128], [129, DEPTH], [1, 129]]))
            t_Baug = stat.tile([128, DEPTH, 129], bf, tag="t_Baug")
            nc.sync.dma_start(t_Baug[:],
                              bv("Baug", [[516, 128], [129, DEPTH], [1, 129]]))
            t_Caug = stat.tile([128, DEPTH, 129], bf, tag="t_Caug")
            nc.sync.dma_start(t_Caug[0:48, :, :],
                              bv("C", [[516, 48], [129, DEPTH], [1, 129]]))
            nc.sync.dma_start(t_Caug[64:112, :, :],
                              bv("C", [[516, 48], [129, DEPTH], [1, 129]]))
            t_We2 = stat.tile([128, DEPTH, 128], bf, tag="t_We2")
            nc.sync.dma_start(t_We2[:],
                              bv("We2", [[512, 128], [128, DEPTH], [1, 128]]))
            t_Watt = stat.tile([128, DEPTH, 1], bf, tag="t_Watt")
            nc.sync.dma_start(t_Watt[:],
                              bv("Watt", [[DEPTH, 128], [1, DEPTH], [1, 1]]))
            t_N1 = stat.tile([128, DEPTH, 2, 129], bf, tag="t_N1")
            nc.sync.dma_start(t_N1[:],
                              bv("N1", [[1032, 128], [258, DEPTH], [129, 2], [1, 129]]))
            t_Wn2 = stat.tile([128, DEPTH, 128], bf, tag="t_Wn2")
            nc.sync.dma_start(t_Wn2[:],
                              bv("Wn2", [[512, 128], [128, DEPTH], [1, 128]]))
            t_Win = stat.tile([32, 128], bf, tag="t_Win")
            nc.sync.dma_start(t_Win[:], bv("Win", [[128, 32], [1, 128]]))
            t_Woe = stat.tile([128, 64], bf, tag="t_Woe")
            nc.sync.dma_start(t_Woe[:], bv("Woe", [[64, 128], [1, 64]]))
            t_pool = stat.tile([128, 4], bf, tag="t_pool")
            nc.sync.dma_start(t_pool[:], bv("pool", [[4, 128], [1, 4]]))
            t_Wfhl = stat.tile([64, 2], bf, tag="t_Wfhl")
            nc.sync.dma_start(t_Wfhl[:], bv("Wfhl", [[2, 64], [1, 2]]))
            t_Wf = stat.tile([64, 1], f32, tag="t_Wf")
            nc.vector.tensor_tensor(out=t_Wf[:], in0=t_Wfhl[:, 0:1],
                                    in1=t_Wfhl[:, 1:2], op=ALU.add)
            t_eps = stat.tile([128, 1], f32, tag="t_eps")
            nc.vector.memset(t_eps[:], EPS)

            # ---------- build col-gather one-hot on device ----------
            # iota over partitions (f32, ints exact)
            t_ii = stat.tile([128, 1], i32, tag="t_ii")
            nc.gpsimd.iota(t_ii[:], pattern=[[0, 1]], base=0,
                           channel_multiplier=1)
            t_if = stat.tile([128, 1], f32, tag="t_if")
            nc.vector.tensor_copy(t_if[:], t_ii[:])
            # band selector E4[r, p] = 1 iff p//32 == r
            t_e4i = stat.tile([4, 128], i32, tag="t_e4i")
            nc.gpsimd.iota(t_e4i[:], pattern=[[1, 128]], base=0,
                           channel_multiplier=-32)
            t_e4f = stat.tile([4, 128], f32, tag="t_e4f")
            nc.vector.tensor_copy(t_e4f[:], t_e4i[:])
            t_ge = stat.tile([4, 128], f32, tag="t_ge")
            nc.vector.tensor_scalar(out=t_ge[:], in0=t_e4f[:], scalar1=0.0,
                                    scalar2=None, op0=ALU.is_ge)
            t_le = stat.tile([4, 128], f32, tag="t_le")
            nc.vector.tensor_scalar(out=t_le[:], in0=t_e4f[:], scalar1=31.0,
                                    scalar2=None, op0=ALU.is_le)
            t_E4 = stat.tile([4, 128], bf, tag="t_E4")
            nc.vector.tensor_tensor(out=t_E4[:], in0=t_ge[:], in1=t_le[:],
                                    op=ALU.mult)
            t_oh = stat.tile([128, 8192], bf, tag="t_oh")
            iota_bc = bass.AP(tensor=t_if[:].tensor, offset=t_if[:].offset,
                              ap=[t_if[:].ap[0], [0, 512]])
            for r in range(16):
                pm = ps3.tile([128, 512], f32, tag="aggatt")
                nc.tensor.matmul(pm[:], lhsT=t_E4[0:4, :],
                                 rhs=t_v4[0:4, 512 * r:512 * r + 512],
                                 start=True, stop=True)
                nc.vector.tensor_tensor(out=t_oh[:, 512 * r:512 * r + 512],
                                        in0=pm[:], in1=iota_bc, op=ALU.is_equal)

            # ---------- build R/S gather patterns on device ----------
            # R[p, 128*hs+e] = 1 iff p%32 == 16*hs + e//8
            t_pm32 = stat.tile([128, 1], i32, tag="t_pm32")
            for qd in range(4):
                nc.gpsimd.iota(t_pm32[32 * qd:32 * qd + 32, :], pattern=[[0, 1]],
                               base=0, channel_multiplier=1)
            t_pm32f = stat.tile([128, 1], f32, tag="t_pm32f")
            nc.vector.tensor_copy(t_pm32f[:], t_pm32[:])
            t_vRi = stat.tile([1, 256], i32, tag="t_vRi")
            nc.gpsimd.iota(t_vRi[:], pattern=[[16, 2], [1, 16], [0, 8]],
                           base=0, channel_multiplier=0)
            t_vRb = stat.tile([1, 256], bf, tag="t_vRb")
            nc.vector.tensor_copy(t_vRb[:], t_vRi[:])
            # S[p, c] = 1 iff p//8 == (c%32) - 16*((c//32)%2)
            t_rSi = stat.tile([1, 256], i32, tag="t_rSi")
            nc.gpsimd.iota(t_rSi[:], pattern=[[0, 4], [-16, 2], [1, 32]],
                           base=0, channel_multiplier=0)
            t_rSb = stat.tile([1, 256], bf, tag="t_rSb")
            nc.vector.tensor_copy(t_rSb[:], t_rSi[:])
            t_ones1 = stat.tile([1, 128], bf, tag="t_ones1")
            nc.vector.memset(t_ones1[:], 1.0)
            pm32_bc = bass.AP(tensor=t_pm32f[:].tensor, offset=t_pm32f[:].offset,
                              ap=[t_pm32f[:].ap[0], [0, 256]])
            if_bc = bass.AP(tensor=t_if[:].tensor, offset=t_if[:].offset,
                            ap=[t_if[:].ap[0], [0, 256]])
            pR = ps3.tile([128, 512], f32, tag="aggatt")
            nc.tensor.matmul(pR[:, 0:256], lhsT=t_ones1[:], rhs=t_vRb[:],
                             start=True, stop=True)
            nc.vector.tensor_tensor(out=t_R[:], in0=pR[:, 0:256], in1=pm32_bc,
                                    op=ALU.is_equal)
            pS = ps3.tile([128, 512], f32, tag="aggatt")
            nc.tensor.matmul(pS[:, 0:256], lhsT=t_ones1[:], rhs=t_rSb[:],
                             start=True, stop=True)
            tmpS = stat.tile([128, 256], f32, tag="tmpS")
            nc.vector.scalar_tensor_tensor(tmpS[:], in0=pS[:, 0:256], scalar=-8.0,
                                           in1=if_bc, op0=ALU.mult, op1=ALU.add)
            tSa = stat.tile([128, 256], f32, tag="tSa")
            nc.vector.tensor_scalar(out=tSa[:], in0=tmpS[:], scalar1=0.0,
                                    scalar2=None, op0=ALU.is_ge)
            tSb = stat.tile([128, 256], f32, tag="tSb")
            nc.vector.tensor_scalar(out=tSb[:], in0=tmpS[:], scalar1=7.0,
                                    scalar2=None, op0=ALU.is_le)
            nc.vector.tensor_tensor(out=t_S[:], in0=tSa[:], in1=tSb[:],
                                    op=ALU.mult)

            # ---------- gaussian smearing on device ----------
            # negoff/coeff as f32 (bf16 hi+lo summed), per-partition scalars
            t_negoff = stat.tile([32, 1], f32, tag="t_negoff")
            nc.vector.tensor_tensor(out=t_negoff[:], in0=t_gshl[:, 0:1],
                                    in1=t_gshl[:, 1:2], op=ALU.add)
            t_coeff = stat.tile([32, 1], f32, tag="t_coeff")
            nc.vector.tensor_tensor(out=t_coeff[:], in0=t_gshl[:, 2:3],
                                    in1=t_gshl[:, 3:4], op=ALU.add)
            t_ones2 = stat.tile([2, 32], bf, tag="t_ones2")
            nc.vector.memset(t_ones2[:], 1.0)
            for hf in range(2):
                for cc in range(32):
                    c0 = 16384 * hf + 512 * cc
                    td = sml.tile([2, 512], bf, tag="td")
                    nc.sync.dma_start(td[:], bv("dhl", [[NEc, 2], [1, 512]], c0))
                    pd = ps3.tile([128, 512], f32, tag="aggatt")
                    nc.tensor.matmul(pd[0:32, :], lhsT=t_ones2[:],
                                     rhs=td[:],
                                     start=True, stop=True)
                    dd2 = sml.tile([32, 512], f32, tag="dd2")
                    nc.scalar.activation(dd2[:], pd[0:32, :], AF.Square,
                                         bias=t_negoff[:], scale=1.0)
                    nc.scalar.activation(
                        t_ea[64 * hf:64 * hf + 32, 512 * cc:512 * cc + 512],
                        dd2[:], AF.Exp, scale=t_coeff[:])

            # ---------- persistent node state ----------
            hh_f = hhp.tile([128, NLc], f32, tag="hh_f")
            hh_b = hhp.tile([128, NLc], bf, tag="hh_b")
            agg_fm = hhp.tile([128, NLc], bf, tag="agg_fm")
            nm_fm = hhp.tile([128, NLc], bf, tag="nm_fm")
            nodeA = hhp.tile([128, 32, 129], bf, tag="nodeA")
            nodeB = hhp.tile([128, 32, 129], bf, tag="nodeB")
            att_em = hhp.tile([128, NCHUNK], f32, tag="att_em")

            # ---------- prologue: hh0 = [h|emb] @ Win ----------
            for nb in range(8):
                p = ps2.tile([128, 2, 512], f32, tag="v2")
                nc.tensor.matmul(p[:, 0, :], lhsT=t_Win[:], rhs=t_in[:, 512 * nb:512 * nb + 512],
                                 start=True, stop=True)
                nc.scalar.activation(hh_f[:, 512 * nb:512 * nb + 512], p[:, 0, :],
                                     AF.Copy)
                nc.vector.tensor_copy(hh_b[:, 512 * nb:512 * nb + 512], p[:, 0, :])

            # ---------- layers ----------
            for l in range(DEPTH):
                # nodeA/nodeB (node-major, 129 cols incl aug-mean)
                for nb in range(32):
                    pn = ps1.tile([128, 2, 512], f32, tag="m1pre")
                    nc.tensor.matmul(pn[:, 0, 0:129], lhsT=hh_b[:, 128 * nb:128 * nb + 128],
                                     rhs=t_Aaug[:, l, :], start=True, stop=True)
                    nc.tensor.matmul(pn[:, 1, 0:129], lhsT=hh_b[:, 128 * nb:128 * nb + 128],
                                     rhs=t_Baug[:, l, :], start=True, stop=True)
                    nc.scalar.activation(nodeA[:, nb, :], pn[:, 0, 0:129], AF.Copy)
                    nc.vector.tensor_copy(nodeB[:, nb, :], pn[:, 1, 0:129])

                for g in range(NGRP):
                    # ---- m1_pre: process in 2 halves of 4 chunks (2 psum tiles) ----
                    m1_em = stg.tile([128, 1024], bf, tag="m1_em")
                    m1_fm = stg.tile([128, 1024], bf, tag="m1_fm")
                    for half in range(2):
                        pts = []
                        for hh2 in range(2):
                            pt = ps1.tile([128, 2, 512], f32, tag="m1pre")
                            pts.append(pt)
                        mv4 = sml.tile([128, 4, 2], f32, tag="mv4")
                        st4 = sml.tile([128, 4, 6], f32, tag="st4")
                        for jj in range(4):
                            j = 4 * half + jj
                            c = 8 * g + j
                            L = c // 2
                            base = 32 * (L % 4)
                            hs = c % 2
                            eh = 0 if c < 128 else 1
                            pt = pts[jj // 2]
                            sl = pt[:, jj % 2, 0:129]
                            nc.tensor.matmul(sl, lhsT=t_R[base:base + 32, 128 * hs:128 * hs + 128],
                                             rhs=nodeA[base:base + 32, L // 4, :],
                                             start=True, stop=False, tile_position=(base, 0))
                            ohf = 128 * (2 * (c // 8) + hs)
                            nc.tensor.matmul(sl, lhsT=t_oh[base:base + 32, ohf:ohf + 128],
                                             rhs=nodeB[base:base + 32, L // 4, :],
                                             start=False, stop=False, tile_position=(base, 0))
                            nc.tensor.matmul(sl, lhsT=t_ea[64 * eh:64 * eh + 48, 128 * (c % 128):128 * (c % 128) + 128],
                                             rhs=t_Caug[64 * eh:64 * eh + 48, l, :],
                                             start=False, stop=True, tile_position=(64 * eh, 0))
                            nc.vector.bn_stats(st4[:, jj, :], pt[:, jj % 2, 0:128])
                            nc.vector.bn_aggr(mv4[:, jj, :], st4[:, jj, :])
                        rstd4 = sml.tile([128, 4], f32, tag="rstd4")
                        nmr4 = sml.tile([128, 4], f32, tag="nmr4")
                        nc.scalar.activation(rstd4[:], mv4[:, :, 1], AF.Sqrt, bias=t_eps[:], scale=1.0)
                        nc.vector.reciprocal(rstd4[:], rstd4[:])
                        nc.vector.scalar_tensor_tensor(nmr4[:], in0=mv4[:, :, 0], scalar=-1.0,
                                                       in1=rstd4[:], op0=ALU.mult, op1=ALU.mult)
                        for jj in range(4):
                            j = 4 * half + jj
                            pt = pts[jj // 2]
                            nc.scalar.activation(m1_em[:, 128 * j:128 * j + 128], pt[:, jj % 2, 0:128],
                                                 AF.Silu, bias=nmr4[:, jj:jj + 1], scale=rstd4[:, jj:jj + 1])
                            nc.sync.dma_start_transpose(m1_fm[:, 128 * j:128 * j + 128],
                                                        m1_em[:, 128 * j:128 * j + 128])
                    # We2 -> v2 (feature-major) + SiLU -> mij_fm bf16
                    pv2 = ps2.tile([128, 2, 512], f32, tag="v2")
                    nc.tensor.matmul(pv2[:, 0, :], lhsT=t_We2[:, l, :], rhs=m1_fm[:, 0:512],
                                     start=True, stop=True)
                    nc.tensor.matmul(pv2[:, 1, :], lhsT=t_We2[:, l, :], rhs=m1_fm[:, 512:1024],
                                     start=True, stop=True)
                    mij_fm = stg.tile([128, 1024], bf, tag="mij_fm")
                    nc.scalar.activation(mij_fm[:], pv2[:].rearrange("p a b -> p (a b)"), AF.Silu)
                    # att: edge-major [128,1] per chunk via mij_fm as lhsT
                    patt = ps3.tile([128, 512], f32, tag="aggatt")
                    for j in range(8):
                        nc.tensor.matmul(patt[:, j:j + 1], lhsT=mij_fm[:, 128 * j:128 * j + 128],
                                         rhs=t_Watt[:, l, :], start=True, stop=True)
                    nc.scalar.activation(att_em[:, 8 * g:8 * g + 8], patt[:, 0:8], AF.Sigmoid)
                    # S*att (bf16) via bcast-TT
                    satt = stg.tile([128, 256], bf, tag="satt")
                    att_bc = bass.AP(tensor=att_em[:].tensor, offset=att_em[:, 8 * g:8 * g + 8].offset,
                                     ap=[att_em[:].ap[0], [1, 8], [0, 32]])
                    nc.vector.tensor_tensor(out=satt[:].rearrange("p (a b) -> p a b", a=8),
                                            in0=t_S[:].rearrange("p (a b) -> p a b", a=8),
                                            in1=att_bc, op=ALU.mult)
                    # mij back to edge-major
                    mij_em = stg.tile([128, 1024], bf, tag="mij_em")
                    for j in range(8):
                        nc.sync.dma_start_transpose(mij_em[:, 128 * j:128 * j + 128],
                                                    mij_fm[:, 128 * j:128 * j + 128])
                    # gated segment-sum -> node-major agg [128 nodes, 128]
                    pagg = ps3.tile([128, 512], f32, tag="aggatt")
                    for j in range(8):
                        nc.tensor.matmul(pagg[32 * (j // 2):32 * (j // 2) + 32, 0:128],
                                         lhsT=satt[:, 32 * j:32 * j + 32],
                                         rhs=mij_em[:, 128 * j:128 * j + 128],
                                         start=(j % 2 == 0), stop=(j % 2 == 1),
                                         tile_position=(0, 32 * (j // 2)))
                    # evac agg (node-major bf16) then transpose to feature-major
                    agg_nm = stg.tile([128, 128], bf, tag="agg_nm")
                    nc.scalar.activation(agg_nm[:], pagg[:, 0:128], AF.Copy)
                    nc.sync.dma_start_transpose(agg_fm[:, 128 * g:128 * g + 128], agg_nm[:])

                # ---- node MLP ----
                for nb in range(16):
                    pn = ps1.tile([128, 2, 512], f32, tag="m1pre")
                    mv2 = sml.tile([128, 2, 2], f32, tag="mv2")
                    st2 = sml.tile([128, 2, 6], f32, tag="st2")
                    for s in range(2):
                        cb = 2 * nb + s
                        sl = pn[:, s, 0:129]
                        nc.tensor.matmul(sl, lhsT=hh_b[:, 128 * cb:128 * cb + 128],
                                         rhs=t_N1[:, l, 0, :], start=True, stop=False)
                        nc.tensor.matmul(sl, lhsT=agg_fm[:, 128 * cb:128 * cb + 128],
                                         rhs=t_N1[:, l, 1, :], start=False, stop=True)
                        nc.vector.bn_stats(st2[:, s, :], pn[:, s, 0:128])
                        nc.vector.bn_aggr(mv2[:, s, :], st2[:, s, :])
                    rstd2 = sml.tile([128, 2], f32, tag="rstd2")
                    nmr2 = sml.tile([128, 2], f32, tag="nmr2")
                    nc.scalar.activation(rstd2[:], mv2[:, :, 1], AF.Sqrt, bias=t_eps[:], scale=1.0)
                    nc.vector.reciprocal(rstd2[:], rstd2[:])
                    nc.vector.scalar_tensor_tensor(nmr2[:], in0=mv2[:, :, 0], scalar=-1.0,
                                                   in1=rstd2[:], op0=ALU.mult, op1=ALU.mult)
                    nm_nm = stg.tile([128, 256], bf, tag="nm_nm")
                    for s in range(2):
                        cb = 2 * nb + s
                        nc.scalar.activation(nm_nm[:, 128 * s:128 * s + 128], pn[:, s, 0:128],
                                             AF.Silu, bias=nmr2[:, s:s + 1], scale=rstd2[:, s:s + 1])
                        nc.sync.dma_start_transpose(nm_fm[:, 128 * cb:128 * cb + 128],
                                                    nm_nm[:, 128 * s:128 * s + 128])
                # hh update: hh += nm @ Wn2
                for nb in range(8):
                    pu = ps2.tile([128, 2, 512], f32, tag="v2")
                    nc.tensor.matmul(pu[:, 0, :], lhsT=t_Wn2[:, l, :],
                                     rhs=nm_fm[:, 512 * nb:512 * nb + 512], start=True, stop=True)
                    nc.vector.tensor_add(hh_f[:, 512 * nb:512 * nb + 512],
                                         hh_f[:, 512 * nb:512 * nb + 512], pu[:, 0, :])
                    nc.vector.tensor_copy(hh_b[:, 512 * nb:512 * nb + 512],
                                          hh_f[:, 512 * nb:512 * nb + 512])

            # ---------- epilogue: ho = hh @ Woe, ligand mean-pool, @ Wf ----------
            pooled_ps = ps3.tile([128, 512], f32, tag="aggatt")
            for nb in range(32):
                ph = ps1.tile([128, 2, 512], f32, tag="m1pre")
                nc.tensor.matmul(ph[:, 0, 0:64], lhsT=hh_b[:, 128 * nb:128 * nb + 128],
                                 rhs=t_Woe[:], start=True, stop=True)
                ho_nm = stg.tile([128, 64], bf, tag="ho_nm")
                nc.scalar.activation(ho_nm[:], ph[:, 0, 0:64], AF.Copy)
                nc.tensor.matmul(pooled_ps[0:64, 4 * nb:4 * nb + 4], lhsT=ho_nm[:],
                                 rhs=t_pool[:], start=True, stop=True)
            pooled_sb = stat.tile([64, 128], f32, tag="pooled_sb")
            nc.vector.tensor_copy(pooled_sb[:], pooled_ps[0:64, 0:128])
            pfin = ps3.tile([128, 512], f32, tag="aggatt")
            nc.tensor.matmul(pfin[0:1, 0:128], lhsT=t_Wf[:], rhs=pooled_sb[:],
                             start=True, stop=True)
            out_sb = stat.tile([1, 128], f32, tag="out_sb")
            nc.vector.tensor_copy(out_sb[:], pfin[0:1, 0:128])
            nc.sync.dma_start(d_out[:], out_sb[:])

    nc.compile()
    return nc


def _get_runner():
    """Build the program and a cached jitted shard_map executor once."""
    if "run" in _COMPILED:
        return _COMPILED["run"]
    import jax
    from jax.sharding import Mesh, PartitionSpec
    from jax.experimental.shard_map import shard_map
    from concourse import mybir
    from concourse.bass2jax import (_bass_exec_p, install_neuronx_cc_hook,
                                    partition_id_tensor)

    nc = _build_program()
    _COMPILED["prog"] = nc
    install_neuronx_cc_hook()

    partition_name = (nc.partition_id_tensor.name
                      if nc.partition_id_tensor else None)
    in_names, out_names, out_avals = [], [], []
    for alloc in nc.m.functions[0].allocations:
        if not isinstance(alloc, mybir.MemoryLocationSet):
            continue
        name = alloc.memorylocations[0].name
        if alloc.kind == "ExternalInput":
            if name != partition_name:
                in_names.append(name)
        elif alloc.kind == "ExternalOutput":
            out_names.append(name)
            out_avals.append(jax.core.ShapedArray(
                tuple(alloc.tensor_shape), mybir.dt.np(alloc.dtype)))
    n_params = len(in_names)
    n_outs = len(out_avals)
    in_names_all = list(in_names) + out_names
    if partition_name is not None:
        in_names_all.append(partition_name)
    donate = tuple(range(n_params, n_params + n_outs))

    def _body(*args):
        operands = list(args)
        if partition_name is not None:
            operands.append(partition_id_tensor())
        outs = _bass_exec_p.bind(
            *operands, out_avals=tuple(out_avals), in_names=tuple(in_names_all),
            out_names=tuple(out_names), lowering_input_output_aliases=(),
            sim_require_finite=True, sim_require_nnan=True, nc=nc)
        return tuple(outs)

    devices = jax.devices()[:NCORES]
    assert len(devices) == NCORES
    mesh = Mesh(np.asarray(devices), ("core",))
    in_specs = (PartitionSpec("core"),) * (n_params + n_outs)
    out_specs = (PartitionSpec("core"),) * n_outs
    sharded = jax.jit(
        shard_map(_body, mesh=mesh, in_specs=in_specs, out_specs=out_specs,
                  check_rep=False),
        keep_unused=True)
    out_shapes = [(NCORES * a.shape[0],) + tuple(a.shape[1:]) for a in out_avals]
    out_dtypes = [a.dtype for a in out_avals]
    from jax.sharding import NamedSharding
    zsh = NamedSharding(mesh, PartitionSpec("core"))
    zdev = [jax.device_put(np.zeros(s, d), zsh)
            for s, d in zip(out_shapes, out_dtypes)]

    def run(blob, blob8):
        """Full dispatch: host->device upload of both blobs, execute on 8
        cores, device->host readback."""
        outs = sharded(blob, blob8, *zdev)
        return np.asarray(outs[0]).reshape(NCORES, LIGc)

    _COMPILED["run"] = run
    return run


# chunk index per (band, block): c = 8*(b//2) + 2*band + (b%2)
_BAND = np.arange(4)[:, None]
_BLK = np.arange(64)[None, :]
_CMAT = 8 * (_BLK // 2) + 2 * _BAND + (_BLK % 2)   # [4, 64]


def _prep_inputs(x, h, t, edges, t_bond, batch_ligand, time_emb_table,
                 W_in, gcl_We1, gcl_Wn1, gcl_We2, gcl_Watt, gcl_Wn2,
                 W_oe, W_f):
    """Host-side sharding + packing into one bf16 blob per core."""
    row = np.asarray(edges[0])
    col = np.asarray(edges[1])
    assert np.array_equal(row, np.repeat(np.arange(N), KNN)), "row structure"
    assert np.array_equal(np.asarray(batch_ligand), np.arange(N) // K), "batch structure"
    assert np.all(col // K == row // K), "edges cross ligands"

    # edge attr: per-feature int8-quantized time-bond embedding codes
    # (scales fold into the C weight rows; error ~s/2 = bf16-rounding level)
    sbi = row * (K - 1) + col - (row // K) * K - (row < col).astype(row.dtype)
    tab = np.asarray(time_emb_table).astype(np.float32)           # [1000,16]
    emb_scale = np.abs(tab).max(0) / 127.0                        # [16]
    emb_scale[emb_scale == 0] = 1.0
    code_tab = np.clip(np.rint(tab / emb_scale[None, :]), -127, 127).astype(np.int8)
    emb_code = code_tab[np.asarray(t_bond)[sbi]]                  # [E,16] int8
    xx = np.asarray(x)
    cdiff = xx[row] - xx[col]
    dist = np.clip(np.sqrt((cdiff ** 2).sum(1)), 0.0, 4.0)
    d_hi = dist.astype(np.float32).astype(bf16)
    d_lo = (dist.astype(np.float32) - d_hi.astype(np.float32)).astype(bf16)

    emb_t = np.asarray(time_emb_table)[np.asarray(t)]             # [N,16]
    hin = np.concatenate([np.asarray(h), emb_t], 1)               # [N,32]

    col_loc = (col % K).astype(np.int64)

    def aug(W):  # [K,128] -> [K,129] with col 128 = row-wise mean over outputs
        return np.concatenate([W, W.mean(1, keepdims=True)], 1)

    We1 = np.asarray(gcl_We1)  # [D, 292, 128]
    Wn1 = np.asarray(gcl_Wn1)  # [D, 256, 128]
    Aaug = np.zeros((128, DEPTH, 129), np.float32)
    Baug = np.zeros((128, DEPTH, 129), np.float32)
    Cstk = np.zeros((48, DEPTH, 129), np.float32)
    N1aug = np.zeros((128, DEPTH, 2, 129), np.float32)
    We2s = np.zeros((128, DEPTH, 128), np.float32)
    Watts = np.zeros((128, DEPTH), np.float32)
    Wn2s = np.zeros((128, DEPTH, 128), np.float32)
    for l in range(DEPTH):
        Aaug[:, l, :] = aug(We1[l][0:128])
        Baug[:, l, :] = aug(We1[l][128:256])
        Cfull = aug(We1[l][256:292])
        Cstk[0:20, l, :] = Cfull[16:36]
        Cstk[32:48, l, :] = Cfull[0:16] * emb_scale[:, None]
        N1aug[:, l, 0, :] = aug(Wn1[l][0:128])
        N1aug[:, l, 1, :] = aug(Wn1[l][128:256] / NORM_FACTOR)
        We2s[:, l, :] = np.asarray(gcl_We2)[l]
        Watts[:, l] = np.asarray(gcl_Watt)[l][:, 0]
        Wn2s[:, l, :] = np.asarray(gcl_Wn2)[l]

    poolpat = np.zeros((128, 4), np.float32)
    for n in range(128):
        poolpat[n, n // 32] = 1.0 / 32.0

    Wf32 = np.asarray(W_f).astype(np.float32).reshape(64, 1)
    Wf_hi = Wf32.astype(bf16)
    Wf_lo = (Wf32 - Wf_hi.astype(np.float32)).astype(bf16)
    Wfhl = np.concatenate([Wf_hi, Wf_lo], 1)                      # [64,2]

    weights = np.concatenate([
        Aaug.ravel(), Baug.ravel(), Cstk.ravel(), We2s.ravel(),
        Watts.ravel(), N1aug.ravel(), Wn2s.ravel(),
        np.asarray(W_in).astype(np.float32).ravel(),
        np.asarray(W_oe).astype(np.float32).ravel(),
        poolpat.ravel(), Wfhl.astype(np.float32).ravel(),
    ]).astype(bf16)
    assert _OFF["Aaug"] + weights.size == TOTAL

    negoff = (-GS_OFFSET).astype(np.float32)
    no_hi = negoff.astype(bf16)
    no_lo = (negoff - no_hi.astype(np.float32)).astype(bf16)
    co_hi = GS_COEFF.astype(bf16)
    co_lo = (GS_COEFF - co_hi.astype(np.float32)).astype(bf16)
    gs = np.zeros((32, 4), bf16)
    gs[:NG] = np.stack([no_hi, no_lo, co_hi, co_lo], 1)           # [20,4]

    blob = np.empty((NCORES, TOTAL), bf16)
    blob8 = np.empty((NCORES, TOTAL8), np.int8)
    blob[:, _OFF["gs"]:_OFF["gs"] + 4 * 32] = gs.ravel()
    blob[:, _OFF["Aaug"]:] = weights
    for ci in range(NCORES):
        n0 = ci * NLc
        e0 = ci * NEc
        blob8[ci, _OFF8["emb8"]:_OFF8["emb8"] + 16 * NEc] = \
            np.ascontiguousarray(emb_code[e0:e0 + NEc].T).ravel()
        blob[ci, _OFF["dhl"]:_OFF["dhl"] + NEc] = d_hi[e0:e0 + NEc]
        blob[ci, _OFF["dhl"] + NEc:_OFF["dhl"] + 2 * NEc] = d_lo[e0:e0 + NEc]
        cl = col_loc[e0:e0 + NEc].reshape(NCHUNK, 128)
        v4 = (32 * _BAND[:, :, None] + cl[_CMAT]).reshape(4, 8192)
        blob8[ci, _OFF8["v48"]:_OFF8["v48"] + 4 * 8192] = \
            v4.astype(np.int8).ravel()
        blob[ci, _OFF["infm"]:_OFF["infm"] + 32 * NLc] = \
            np.ascontiguousarray(hin[n0:n0 + NLc].T).astype(bf16).ravel()
    return blob, blob8


def kernel(x, h, t, edges, t_bond, batch_ligand, num_atoms_per_ligand,
           num_ligands, time_emb_table, W_in, b_in, gcl_We1, gcl_be1, gcl_g1,
           gcl_bt1, gcl_We2, gcl_be2, gcl_Watt, gcl_batt, gcl_Wn1, gcl_bn1,
           gcl_g2, gcl_bt2, gcl_Wn2, gcl_bn2, W_oe, b_oe, W_f, b_f):
    # all biases zero / gains one in this model family; verify then fold away
    for z in (b_in, gcl_be1, gcl_bt1, gcl_be2, gcl_batt, gcl_bn1, gcl_bt2,
              gcl_bn2, b_oe, b_f):
        assert np.abs(np.asarray(z)).max() == 0.0, "nonzero bias unsupported"
    for o in (gcl_g1, gcl_g2):
        assert np.abs(np.asarray(o) - 1.0).max() == 0.0, "non-unit LN gain"
    assert int(num_atoms_per_ligand) == K and int(num_ligands) == N_LIG

    run = _get_runner()
    blob, blob8 = _prep_inputs(x, h, t, edges, t_bond, batch_ligand,
                               time_emb_table, W_in, gcl_We1, gcl_Wn1,
                               gcl_We2, gcl_Watt, gcl_Wn2, W_oe, W_f)
    out = run(blob, blob8)
    return out.reshape(-1).astype(np.float32)
